# revision 11
# baseline (speedup 1.0000x reference)
"""DensePICNN forward pass on 8 TRN2 NeuronCores (Bass/Tile).

Strategy
--------
Pure data parallel: batch N=262144 split 8 ways (32768/core); weights
replicated. Within a core, samples run feature-major in "pair-units" of
2048: half A on SBUF partitions 0-63, half B on 64-127, so elementwise ops
use all 128 lanes and K=64 matmuls row-pack A/B into the 128-row PE array.

Host folds (fp64): actnorm scales, softplus(W) of PosLinear, gains, bias
shifts, the elu(cc)+1 reparameterization (cc' = cc+1; downstream biases get
-rowsum(W) corrections). Matmul operands bf16, PSUM fp32.

softplus(v) = Ln(Exp(v + b) + 1)   (one ACT table set)
elu(a) + 1  = max(a + 1, exp(min(a, 0)))   (exact, by convexity)

x-replication for the (x*cx) products is done on-device by scatter matmuls
(REP blocks) so the input stream stays small. Exactly 4 DMAs total (weights,
2 input halves, 1 output) keeps the kernel-tail drain within the 8-wait ISA
limit (3 engine sems + 4 DMA lanes = 7).
"""
import numpy as np
import ml_dtypes
from contextlib import ExitStack

import concourse.bass as bass
import concourse.tile as tile
from concourse import bacc
import concourse.mybir as mybir
from concourse.bass_utils import run_bass_kernel_spmd

AF = mybir.ActivationFunctionType
OP = mybir.AluOpType
BF = mybir.dt.bfloat16
F32 = mybir.dt.float32
bf16 = ml_dtypes.bfloat16

# ---- problem constants (kernel.py must be self-contained) ----
N, DIM, DIMC, DIMH, L = 262144, 8, 8, 64, 4
NCORES = 8
NC = N // NCORES            # 32768 samples per core
F = 1024                    # free-dim columns per pair-unit
H = F // 2                  # psum-bank column split
UNITS = NC // (2 * F)       # 16 pair-units per core
HALF = NC // 2              # 16384 = UNITS * F

CZ_SHIFT0 = float(np.exp(-1.0))
CZ_SHIFT_LAST = float(np.log(np.e - 1.0))

CHUNKS = [(j, k) for j in range(L) for k in range(j + 1)]      # 10 (j, k)
CIDX = {jk: i for i, jk in enumerate(CHUNKS)}
SJ = [2, 3, 0, 1]           # B-side xx/xcx row-group stagger: row 32*SJ[j]

# wts column layout (bf16)
MM1_C = 0                   # [128] z0 cols 0:64, cc cols 64:128 (4 row-bases)
CZ_C = 128                  # [640] 10 chunks of 64
CX_C = 768                  # [128] cx block, M=128 sparse
XA_C = 896                  # [64]  Wx lhsT, rows 32j+r
XB_C = 960                  # [64]  Wx lhsT, rows 32*SJ[j]+r
PRE_C = 1024                # [580] pre-chain pos/wcc blocks
REPA_C = 1604               # [128] x-scatter for A (rows {0-7, 64-71})
REPB_C = 1732               # [128] x-scatter for B (rows {32-39, 96-103})
BIAS_C = 1860               # [42]  21 fp32 bias columns, bitcast view
TOTW = BIAS_C + 42

# bias column indices (fp32 view)
B_Z0, B_CC, B_CC1 = 0, 1, 2
B_CZ = 3                    # 3..12
B_ZJ = 13                   # 13..15 (z1..z3)
B_FIN = 16
B_CXA, B_CXB = 17, 18
B_ONE, B_ZERO = 19, 20
BCOLS = 21


def _pre_off(j, k):
    base = {0: 0, 1: 128, 2: 320, 3: 576}[j]
    return PRE_C + base + (64 * k if j < 3 else k)


def _wcc_off(j):
    return PRE_C + {0: 64, 1: 256, 2: 512}[j]


def fold_weights(inp):
    """Host fp64 folding -> single packed [128, TOTW] bf16 array (biases
    stored bitwise as fp32 in the last 38 bf16 columns)."""
    g = {k: (np.asarray(v, np.float64) if not isinstance(v, list)
             else [np.asarray(a, np.float64) for a in v]) for k, v in inp.items()}
    sp = lambda v: np.logaddexp(0, v)

    wts = np.zeros((128, BIAS_C), np.float64)
    bias = np.zeros((128, BCOLS), np.float64)

    s0 = np.exp(g['an0_w'])
    W1z = g['Wz_ws'][0] * s0[:, None]
    b1z = g['Wz_bs'][0] * s0 + g['an0_b']
    sc = np.exp(g['anc_w'])
    W1c = g['Wc_w'] * sc[:, None]
    b1c = g['Wc_b'] * sc + g['anc_b']

    for base in (0, 32, 64, 96):
        wts[base + 0:base + 8, MM1_C + 0:MM1_C + 64] = W1z.T
        wts[base + 8:base + 16, MM1_C + 64:MM1_C + 128] = W1c.T
    for half in (slice(0, 64), slice(64, 128)):
        bias[half, B_Z0] = b1z
        bias[half, B_CC] = b1c
        bias[half, B_CC1] = b1c + 1.0

    # REP scatter blocks: psum row 32j+r (A) / 32*SJ[j]+r (B) <- x feat r
    for j in range(L):
        for r in range(DIM):
            for base in (0, 64):
                wts[base + r, REPA_C + 32 * j + r] = 1.0
            for base in (32, 96):
                wts[base + r, REPB_C + 32 * SJ[j] + r] = 1.0

    # cz chunks
    for (j, k) in CHUNKS:
        ci = CIDX[(j, k)]
        Wcz = g['Wcz_ws'][j]
        shift = CZ_SHIFT0 if j < L - 1 else CZ_SHIFT_LAST
        bcz = g['Wcz_bs'][j] + shift - Wcz.sum(1)
        Wch = Wcz[64 * k:64 * (k + 1), :]
        for half in (slice(0, 64), slice(64, 128)):
            wts[half, CZ_C + 64 * ci:CZ_C + 64 * (ci + 1)] = Wch.T
            bias[half, B_CZ + ci] = bcz[64 * k:64 * (k + 1)]

    # cx block + bcx bias columns
    for j in range(L):
        Wcx = g['Wcx_ws'][j]
        bcx = g['Wcx_bs'][j] + 1.0 - Wcx.sum(1)
        ra, rb = 32 * j, 32 * SJ[j]
        for r in range(DIM):
            wts[0:64, CX_C + ra + r] = Wcx[r, :]
            wts[64:128, CX_C + rb + r] = Wcx[r, :]
            bias[ra + r, B_CXA] = bcx[r]
            bias[rb + r, B_CXB] = bcx[r]

    # pre-chain weights
    for j in range(L):
        if j < L - 1:
            sjx = np.exp(g['an_ws'][j])
            gain = 1.0 / (DIMH * (j + 1))
            Wp = sp(g['Wz_ws'][j + 1]) * gain * sjx[:, None]
            Wxx = g['Wx_ws'][j] * sjx[:, None]
            Wcc = g['Wcc_ws'][j] * sjx[:, None]
            bpre = ((g['Wz_bs'][j + 1] * gain + g['Wx_bs'][j] + g['Wcc_bs'][j]
                     - g['Wcc_ws'][j].sum(1)) * sjx + g['an_bs'][j])
            for half in (slice(0, 64), slice(64, 128)):
                bias[half, B_ZJ + j] = bpre
                wts[half, _wcc_off(j):_wcc_off(j) + 64] = Wcc.T
        else:
            sL = np.exp(g['an_ws'][L - 1])
            gain = 1.0 / (DIMH * L)
            Wp = sp(g['Wz_ws'][L]) * gain * sL
            Wxx = g['Wx_ws'][L - 1] * sL
            bias[0, B_FIN] = g['an_bs'][L - 1][0]
            bias[64, B_FIN] = g['an_bs'][L - 1][0]
        M = 64 if j < L - 1 else 1
        for k in range(j + 1):
            blk = Wp[:, 64 * k:64 * (k + 1)]
            off = _pre_off(j, k)
            wts[0:64, off:off + M] = blk.T
            wts[64:128, off:off + M] = blk.T
        ra, rb = 32 * j, 32 * SJ[j]
        for r in range(DIM):
            wts[ra + r, XA_C:XA_C + M] = Wxx[:, r]
            wts[rb + r, XB_C:XB_C + M] = Wxx[:, r]

    bias[:, B_ONE] = 1.0
    packed = np.zeros((128, TOTW), bf16)
    packed[:, :BIAS_C] = wts.astype(bf16)
    packed[:, BIAS_C:] = bias.astype(np.float32).view(np.uint16).view(bf16)
    return packed


def pack_inputs(x, c):
    """Per-core xin [128, 8F] bf16: block b holds units (2b, 2b+1); unit at
    row base 64*(u%2): rows +0..7 x[A], +8..15 c[A], +32..39 x[B], +40..47 c[B].
    """
    x = np.asarray(x, np.float32).reshape(NCORES, NC, DIM)
    c = np.asarray(c, np.float32).reshape(NCORES, NC, DIMC)
    xins = []
    for core in range(NCORES):
        xin = np.zeros((128, UNITS // 2 * F), np.float32)
        for u in range(UNITS):
            b, p = divmod(u, 2)
            cols = slice(b * F, (b + 1) * F)
            s0 = u * 2 * F
            ra = 64 * p
            xin[ra + 0:ra + 8, cols] = x[core][s0:s0 + F].T
            xin[ra + 8:ra + 16, cols] = c[core][s0:s0 + F].T
            xin[ra + 32:ra + 40, cols] = x[core][s0 + F:s0 + 2 * F].T
            xin[ra + 40:ra + 48, cols] = c[core][s0 + F:s0 + 2 * F].T
        xins.append(xin.astype(bf16))
    return xins


def build_bass():
    nc = bacc.Bacc("TRN2")
    xin = nc.declare_dram_parameter("xin", [128, UNITS // 2 * F], BF, isOutput=False)
    wtp = nc.declare_dram_parameter("wts", [128, TOTW], BF, isOutput=False)
    out2 = nc.declare_dram_parameter("out2", [128, HALF], BF, isOutput=True)

    with ExitStack() as ctx:
        tc = ctx.enter_context(tile.TileContext(nc))
        wpool = ctx.enter_context(tc.tile_pool(name="w", bufs=1))
        inpool = ctx.enter_context(tc.tile_pool(name="xi", bufs=2))
        epool = ctx.enter_context(tc.tile_pool(name="e", bufs=4))
        zpool = ctx.enter_context(tc.tile_pool(name="z", bufs=2))
        czpool = ctx.enter_context(tc.tile_pool(name="cz", bufs=2))
        qpool = ctx.enter_context(tc.tile_pool(name="q", bufs=2))
        mpool = ctx.enter_context(tc.tile_pool(name="m", bufs=2))
        opool = ctx.enter_context(tc.tile_pool(name="o", bufs=1))
        pspool = ctx.enter_context(tc.tile_pool(name="ps", bufs=4, space="PSUM"))

        wt = wpool.tile([128, TOTW], BF, name="wt")
        nc.sync.dma_start(wt[:], wtp[:])
        bi = wt[:, BIAS_C:TOTW].bitcast(F32)          # [128, 19] fp32 view

        outbuf = opool.tile([128, HALF], BF, name="outbuf")

        def mm(out_ap, lhsT, rhs, start, stop, tp=None):
            for h in (0, H):
                nc.tensor.matmul(out_ap[:, h:h + H], lhsT, rhs[:, h:h + H],
                                 start=start, stop=stop, tile_position=tp,
                                 skip_group_check=True)

        state = {}

        def front(u):
            b, p = divmod(u, 2)
            hf, lb = divmod(b, 4)
            if u % 8 == 0:
                xt = inpool.tile([128, 4 * F], BF, tag="xin", name=f"xin{hf}")
                nc.sync.dma_start(xt[:], xin[:, hf * 4 * F:(hf + 1) * 4 * F])
                state["xt"] = xt
            xc = state["xt"][:, lb * F:(lb + 1) * F]
            rA, rB = 64 * p, 64 * p + 32

            def tpr(row, col):
                return (row, col) if row == 96 else None

            ps_z0 = pspool.tile([128, F], F32, tag="ps", name=f"psz0_{u}")
            mm(ps_z0[0:64, :], wt[rA:rA + 16, MM1_C:MM1_C + 64], xc[rA:rA + 16, :],
               True, True, tpr(rA, 0))
            mm(ps_z0[64:128, :], wt[rB:rB + 16, MM1_C:MM1_C + 64], xc[rB:rB + 16, :],
               True, True, tpr(rB, 64))
            ps_cc = pspool.tile([128, F], F32, tag="ps", name=f"pscc{u}")
            mm(ps_cc[0:64, :], wt[rA:rA + 16, MM1_C + 64:MM1_C + 128],
               xc[rA:rA + 16, :], True, True, tpr(rA, 0))
            mm(ps_cc[64:128, :], wt[rB:rB + 16, MM1_C + 64:MM1_C + 128],
               xc[rB:rB + 16, :], True, True, tpr(rB, 64))

            e0 = epool.tile([128, F], F32, tag="e", name=f"e0_{u}")
            nc.scalar.activation(e0[:], ps_z0[:], AF.Exp, bias=bi[:, B_Z0:B_Z0 + 1])
            zt = [zpool.tile([128, F], BF, tag=f"z{k}", name=f"zt{u}_{k}")
                  for k in range(L)]
            nc.scalar.activation(zt[0][:], e0[:], AF.Ln, bias=bi[:, B_ONE:B_ONE + 1])

            em = epool.tile([128, F], F32, tag="e", name=f"em{u}")
            nc.scalar.activation(em[:], ps_cc[:], AF.Exp, bias=bi[:, B_CC:B_CC + 1])
            em2 = mpool.tile([128, F], F32, tag="m", name=f"em2_{u}")
            nc.vector.tensor_scalar(em2[:], em[:], 1.0, None, OP.min)
            ccp = mpool.tile([128, F], BF, tag="ccp", name=f"ccp{u}")
            nc.vector.scalar_tensor_tensor(ccp[:], ps_cc[:], bi[:, B_CC1:B_CC1 + 1],
                                           em2[:], OP.add, OP.max)

            ps_repA = pspool.tile([128, F], F32, tag="ps", name=f"psra{u}")
            mm(ps_repA[:, :], wt[rA:rA + 16, REPA_C:REPA_C + 128],
               xc[rA:rA + 16, :], True, True, tpr(rA, 0))
            ps_cxA = pspool.tile([128, F], F32, tag="ps", name=f"pscxa{u}")
            mm(ps_cxA[:, :], wt[0:64, CX_C:CX_C + 128], ccp[0:64, :], True, True)
            cxa = mpool.tile([128, F], BF, tag="cxa", name=f"cxa{u}")
            nc.vector.tensor_scalar(cxa[:], ps_cxA[:], bi[:, B_CXA:B_CXA + 1],
                                    None, OP.add)
            xxa = mpool.tile([128, F], BF, tag="xxa", name=f"xxa{u}")
            nc.vector.tensor_mul(xxa[:], cxa[:], ps_repA[:])

            ps_repB = pspool.tile([128, F], F32, tag="ps", name=f"psrb{u}")
            mm(ps_repB[:, :], wt[rB:rB + 16, REPB_C:REPB_C + 128],
               xc[rB:rB + 16, :], True, True, tpr(rB, 0))
            ps_cxB = pspool.tile([128, F], F32, tag="ps", name=f"pscxb{u}")
            mm(ps_cxB[:, :], wt[64:128, CX_C:CX_C + 128], ccp[64:128, :], True, True)
            cxb = mpool.tile([128, F], BF, tag="cxb", name=f"cxb{u}")
            nc.vector.tensor_scalar(cxb[:], ps_cxB[:], bi[:, B_CXB:B_CXB + 1],
                                    None, OP.add)
            xxb = mpool.tile([128, F], BF, tag="xxb", name=f"xxb{u}")
            nc.vector.tensor_mul(xxb[:], cxb[:], ps_repB[:])

            czt = []
            for ci, (j, k) in enumerate(CHUNKS):
                ps_c = pspool.tile([128, F], F32, tag="ps", name=f"psc{u}_{ci}")
                co = CZ_C + 64 * ci
                mm(ps_c[0:64, :], wt[0:64, co:co + 64], ccp[0:64, :], True, True)
                mm(ps_c[64:128, :], wt[64:128, co:co + 64], ccp[64:128, :],
                   True, True)
                ec = epool.tile([128, F], F32, tag="e", name=f"ec{u}_{ci}")
                nc.scalar.activation(ec[:], ps_c[:], AF.Exp,
                                     bias=bi[:, B_CZ + ci:B_CZ + ci + 1])
                ct = czpool.tile([128, F], BF, tag=f"c{ci}", name=f"ct{u}_{ci}")
                nc.scalar.activation(ct[:], ec[:], AF.Ln,
                                     bias=bi[:, B_ONE:B_ONE + 1])
                czt.append(ct)

            # products against z0 (ready now) for all stages
            qts = {}
            for j in range(L):
                ci = CIDX[(j, 0)]
                qt = qpool.tile([128, F], BF, tag=f"q{ci}", name=f"qt{u}_{j}_0")
                nc.vector.tensor_mul(qt[:], zt[0][:], czt[ci][:])
                qts[(j, 0)] = qt
            return dict(u=u, zt=zt, czt=czt, ccp=ccp, xxa=xxa, xxb=xxb, qts=qts)

        def back(st):
            u, zt, czt, ccp, xxa, xxb, qts = (st["u"], st["zt"], st["czt"],
                                              st["ccp"], st["xxa"], st["xxb"],
                                              st["qts"])
            for j in range(L):
                M = 64 if j < L - 1 else 1
                ps_pre = pspool.tile([128, F], F32, tag="ps", name=f"pspre{u}_{j}")
                outA = ps_pre[0:M, :]
                outB = ps_pre[64:64 + M, :]
                ra, rb = 32 * j, 32 * SJ[j]
                mm(outA, wt[ra:ra + 8, XA_C:XA_C + M], xxa[ra:ra + 8, :],
                   True, False, (96, 0) if ra == 96 else None)
                mm(outB, wt[rb:rb + 8, XB_C:XB_C + M], xxb[rb:rb + 8, :],
                   True, False, (96, 64) if rb == 96 else None)
                for k in range(j + 1):
                    qt = qts[(j, k)]
                    off = _pre_off(j, k)
                    last = (j == L - 1) and (k == j)
                    mm(outA, wt[0:64, off:off + M], qt[0:64, :], False, last)
                    mm(outB, wt[64:128, off:off + M], qt[64:128, :], False, last)
                if j < L - 1:
                    wo = _wcc_off(j)
                    mm(outA, wt[0:64, wo:wo + 64], ccp[0:64, :], False, True)
                    mm(outB, wt[64:128, wo:wo + 64], ccp[64:128, :], False, True)
                    ej = epool.tile([128, F], F32, tag="e", name=f"ej{u}_{j}")
                    nc.scalar.activation(ej[:], ps_pre[:], AF.Exp,
                                         bias=bi[:, B_ZJ + j:B_ZJ + j + 1])
                    nc.scalar.activation(zt[j + 1][:], ej[:], AF.Ln,
                                         bias=bi[:, B_ONE:B_ONE + 1])
                    # products against the fresh z_{j+1} for later stages
                    for j2 in range(j + 1, L):
                        ci = CIDX[(j2, j + 1)]
                        qt = qpool.tile([128, F], BF, tag=f"q{ci}",
                                        name=f"qt{u}_{j2}_{j + 1}")
                        nc.vector.tensor_mul(qt[:], zt[j + 1][:], czt[ci][:])
                        qts[(j2, j + 1)] = qt
                else:
                    nc.vector.tensor_scalar(outbuf[:, u * F:(u + 1) * F], ps_pre[:],
                                            bi[:, B_FIN:B_FIN + 1], None, OP.add)

        prev = None
        for u in range(UNITS):
            cur = front(u)
            if prev is not None:
                back(prev)
            prev = cur
        back(prev)

        nc.sync.dma_start(out2[:], outbuf[:])
    _compile_with_pinned_act_tables(nc)
    return nc


def _compile_with_pinned_act_tables(nc):
    """bacc's insert_act_table_loads greedily picks the first table set
    containing each activation function, which alternates Exp->exp_and_others
    and Ln->natural_log, reloading tables (~1.3us) on nearly every ACTIVATE.
    Restrict Exp/Ln to the combined set (contents only -- set ids/order are
    untouched) so one load serves the whole kernel, then restore."""
    import concourse.mybir as _mb
    orig = bacc.get_activation_tables

    def patched(arch):
        tabs = orig(arch)
        out = {}
        for name, funcs in tabs.items():
            if name != "natural_log_exp_and_others":
                funcs = {f for f in funcs
                         if f not in (_mb.ActivationFunctionType.Exp,
                                      _mb.ActivationFunctionType.Ln)}
            out[name] = funcs
        return out

    bacc.get_activation_tables = patched
    try:
        nc.compile()
    finally:
        bacc.get_activation_tables = orig


_CACHED = {}


def _get_bass():
    if "nc" not in _CACHED:
        _CACHED["nc"] = build_bass()
    return _CACHED["nc"]


def run(inputs, trace=False):
    """Returns (out [N, 1] fp32, BassKernelResults)."""
    wts = fold_weights(inputs)
    xins = pack_inputs(inputs["x"], inputs["c"])
    nc = _get_bass()
    in_maps = [dict(xin=xins[core], wts=wts) for core in range(NCORES)]
    res = run_bass_kernel_spmd(nc, in_maps, list(range(NCORES)), trace=trace)
    outs = []
    for core in range(NCORES):
        o2 = res.results[core]["out2"]                     # [128, HALF] bf16
        a = np.asarray(o2[0]).astype(np.float32).reshape(UNITS, F)
        bb = np.asarray(o2[64]).astype(np.float32).reshape(UNITS, F)
        o = np.stack([a, bb], axis=1).reshape(NC)          # [units, 2, F]
        outs.append(o)
    return np.concatenate(outs).reshape(N, 1).astype(np.float32), res


def kernel(**inputs):
    out, _ = run(inputs, trace=False)
    return out


# revision 12
# speedup vs baseline: 1.1508x; 1.1508x over previous
"""DensePICNN forward pass on 8 TRN2 NeuronCores (Bass/Tile).

Strategy
--------
Pure data parallel: batch N=262144 split 8 ways (32768/core); weights
replicated. Within a core, samples run feature-major in "pair-units" of
2048: half A on SBUF partitions 0-63, half B on 64-127, so elementwise ops
use all 128 lanes and K=64 matmuls row-pack A/B into the 128-row PE array.

Host folds (fp64): actnorm scales, softplus(W) of PosLinear, gains, bias
shifts, the elu(cc)+1 reparameterization (cc' = cc+1; downstream biases get
-rowsum(W) corrections). Matmul operands bf16, PSUM fp32.

softplus(v) = Ln(Exp(v + b) + 1)   (one ACT table set)
elu(a) + 1  = max(a + 1, exp(min(a, 0)))   (exact, by convexity)

x-replication for the (x*cx) products is done on-device by scatter matmuls
(REP blocks) so the input stream stays small. Exactly 4 DMAs total (weights,
2 input halves, 1 output) keeps the kernel-tail drain within the 8-wait ISA
limit (3 engine sems + 4 DMA lanes = 7).
"""
import numpy as np
import ml_dtypes
from contextlib import ExitStack

import concourse.bass as bass
import concourse.tile as tile
from concourse import bacc
import concourse.mybir as mybir
from concourse.bass_utils import run_bass_kernel_spmd

AF = mybir.ActivationFunctionType
OP = mybir.AluOpType
BF = mybir.dt.bfloat16
F32 = mybir.dt.float32
bf16 = ml_dtypes.bfloat16

# ---- problem constants (kernel.py must be self-contained) ----
N, DIM, DIMC, DIMH, L = 262144, 8, 8, 64, 4
NCORES = 8
NC = N // NCORES            # 32768 samples per core
F = 1024                    # free-dim columns per pair-unit
H = F // 2                  # psum-bank column split
UNITS = NC // (2 * F)       # 16 pair-units per core
HALF = NC // 2              # 16384 = UNITS * F

CZ_SHIFT0 = float(np.exp(-1.0))
CZ_SHIFT_LAST = float(np.log(np.e - 1.0))

CHUNKS = [(j, k) for j in range(L) for k in range(j + 1)]      # 10 (j, k)
CIDX = {jk: i for i, jk in enumerate(CHUNKS)}
SJ = [2, 3, 0, 1]           # B-side xx/xcx row-group stagger: row 32*SJ[j]

# wts column layout (bf16)
MM1_C = 0                   # [128] z0 cols 0:64, cc cols 64:128 (4 row-bases)
CZ_C = 128                  # [640] 10 chunks of 64
CX_C = 768                  # [128] cx block, M=128 sparse
XA_C = 896                  # [64]  Wx lhsT, rows 32j+r
XB_C = 960                  # [64]  Wx lhsT, rows 32*SJ[j]+r
PRE_C = 1024                # [580] pre-chain pos/wcc blocks
REPA_C = 1604               # [128] x-scatter for A (rows {0-7, 64-71})
REPB_C = 1732               # [128] x-scatter for B (rows {32-39, 96-103})
BIAS_C = 1860               # [42]  21 fp32 bias columns, bitcast view
TOTW = BIAS_C + 42

# bias column indices (fp32 view)
B_Z0, B_CC, B_CC1 = 0, 1, 2
B_CZ = 3                    # 3..12
B_ZJ = 13                   # 13..15 (z1..z3)
B_FIN = 16
B_CXA, B_CXB = 17, 18
B_ONE, B_ZERO = 19, 20
BCOLS = 21


def _pre_off(j, k):
    base = {0: 0, 1: 128, 2: 320, 3: 576}[j]
    return PRE_C + base + (64 * k if j < 3 else k)


def _wcc_off(j):
    return PRE_C + {0: 64, 1: 256, 2: 512}[j]


def fold_weights(inp):
    """Host fp64 folding -> single packed [128, TOTW] bf16 array (biases
    stored bitwise as fp32 in the last 38 bf16 columns)."""
    g = {k: (np.asarray(v, np.float64) if not isinstance(v, list)
             else [np.asarray(a, np.float64) for a in v]) for k, v in inp.items()}
    sp = lambda v: np.logaddexp(0, v)

    wts = np.zeros((128, BIAS_C), np.float64)
    bias = np.zeros((128, BCOLS), np.float64)

    s0 = np.exp(g['an0_w'])
    W1z = g['Wz_ws'][0] * s0[:, None]
    b1z = g['Wz_bs'][0] * s0 + g['an0_b']
    sc = np.exp(g['anc_w'])
    W1c = g['Wc_w'] * sc[:, None]
    b1c = g['Wc_b'] * sc + g['anc_b']

    for base in (0, 32, 64, 96):
        wts[base + 0:base + 8, MM1_C + 0:MM1_C + 64] = W1z.T
        wts[base + 8:base + 16, MM1_C + 64:MM1_C + 128] = W1c.T
    for half in (slice(0, 64), slice(64, 128)):
        bias[half, B_Z0] = b1z
        bias[half, B_CC] = b1c
        bias[half, B_CC1] = b1c + 1.0

    # REP scatter blocks: psum row 32j+r (A) / 32*SJ[j]+r (B) <- x feat r
    for j in range(L):
        for r in range(DIM):
            for base in (0, 64):
                wts[base + r, REPA_C + 32 * j + r] = 1.0
            for base in (32, 96):
                wts[base + r, REPB_C + 32 * SJ[j] + r] = 1.0

    # cz chunks
    for (j, k) in CHUNKS:
        ci = CIDX[(j, k)]
        Wcz = g['Wcz_ws'][j]
        shift = CZ_SHIFT0 if j < L - 1 else CZ_SHIFT_LAST
        bcz = g['Wcz_bs'][j] + shift - Wcz.sum(1)
        Wch = Wcz[64 * k:64 * (k + 1), :]
        for half in (slice(0, 64), slice(64, 128)):
            wts[half, CZ_C + 64 * ci:CZ_C + 64 * (ci + 1)] = Wch.T
            bias[half, B_CZ + ci] = bcz[64 * k:64 * (k + 1)]

    # cx block + bcx bias columns
    for j in range(L):
        Wcx = g['Wcx_ws'][j]
        bcx = g['Wcx_bs'][j] + 1.0 - Wcx.sum(1)
        ra, rb = 32 * j, 32 * SJ[j]
        for r in range(DIM):
            wts[0:64, CX_C + ra + r] = Wcx[r, :]
            wts[64:128, CX_C + rb + r] = Wcx[r, :]
            bias[ra + r, B_CXA] = bcx[r]
            bias[rb + r, B_CXB] = bcx[r]

    # pre-chain weights
    for j in range(L):
        if j < L - 1:
            sjx = np.exp(g['an_ws'][j])
            gain = 1.0 / (DIMH * (j + 1))
            Wp = sp(g['Wz_ws'][j + 1]) * gain * sjx[:, None]
            Wxx = g['Wx_ws'][j] * sjx[:, None]
            Wcc = g['Wcc_ws'][j] * sjx[:, None]
            bpre = ((g['Wz_bs'][j + 1] * gain + g['Wx_bs'][j] + g['Wcc_bs'][j]
                     - g['Wcc_ws'][j].sum(1)) * sjx + g['an_bs'][j])
            for half in (slice(0, 64), slice(64, 128)):
                bias[half, B_ZJ + j] = bpre
                wts[half, _wcc_off(j):_wcc_off(j) + 64] = Wcc.T
        else:
            sL = np.exp(g['an_ws'][L - 1])
            gain = 1.0 / (DIMH * L)
            Wp = sp(g['Wz_ws'][L]) * gain * sL
            Wxx = g['Wx_ws'][L - 1] * sL
            bias[0, B_FIN] = g['an_bs'][L - 1][0]
            bias[64, B_FIN] = g['an_bs'][L - 1][0]
        M = 64 if j < L - 1 else 1
        for k in range(j + 1):
            blk = Wp[:, 64 * k:64 * (k + 1)]
            off = _pre_off(j, k)
            wts[0:64, off:off + M] = blk.T
            wts[64:128, off:off + M] = blk.T
        ra, rb = 32 * j, 32 * SJ[j]
        for r in range(DIM):
            wts[ra + r, XA_C:XA_C + M] = Wxx[:, r]
            wts[rb + r, XB_C:XB_C + M] = Wxx[:, r]

    bias[:, B_ONE] = 1.0
    packed = np.zeros((128, TOTW), bf16)
    packed[:, :BIAS_C] = wts.astype(bf16)
    packed[:, BIAS_C:] = bias.astype(np.float32).view(np.uint16).view(bf16)
    return packed


def pack_inputs(x, c):
    """Per-core xin [128, 8F] bf16: block b holds units (2b, 2b+1); unit at
    row base 64*(u%2): rows +0..7 x[A], +8..15 c[A], +32..39 x[B], +40..47 c[B].
    """
    x = np.asarray(x, np.float32).reshape(NCORES, NC, DIM)
    c = np.asarray(c, np.float32).reshape(NCORES, NC, DIMC)
    xins = []
    for core in range(NCORES):
        xin = np.zeros((128, UNITS // 2 * F), np.float32)
        for u in range(UNITS):
            b, p = divmod(u, 2)
            cols = slice(b * F, (b + 1) * F)
            s0 = u * 2 * F
            ra = 64 * p
            xin[ra + 0:ra + 8, cols] = x[core][s0:s0 + F].T
            xin[ra + 8:ra + 16, cols] = c[core][s0:s0 + F].T
            xin[ra + 32:ra + 40, cols] = x[core][s0 + F:s0 + 2 * F].T
            xin[ra + 40:ra + 48, cols] = c[core][s0 + F:s0 + 2 * F].T
        xins.append(xin.astype(bf16))
    return xins


def build_bass():
    nc = bacc.Bacc("TRN2")
    xin = nc.declare_dram_parameter("xin", [128, UNITS // 2 * F], BF, isOutput=False)
    wtp = nc.declare_dram_parameter("wts", [128, TOTW], BF, isOutput=False)
    out2 = nc.declare_dram_parameter("out2", [128, HALF], BF, isOutput=True)

    with ExitStack() as ctx:
        tc = ctx.enter_context(tile.TileContext(nc))
        wpool = ctx.enter_context(tc.tile_pool(name="w", bufs=1))
        inpool = ctx.enter_context(tc.tile_pool(name="xi", bufs=2))
        epool = ctx.enter_context(tc.tile_pool(name="e", bufs=4))
        zpool = ctx.enter_context(tc.tile_pool(name="z", bufs=2))
        czpool = ctx.enter_context(tc.tile_pool(name="cz", bufs=2))
        qpool = ctx.enter_context(tc.tile_pool(name="q", bufs=2))
        mpool = ctx.enter_context(tc.tile_pool(name="m", bufs=2))
        opool = ctx.enter_context(tc.tile_pool(name="o", bufs=1))
        pspool = ctx.enter_context(tc.tile_pool(name="ps", bufs=4, space="PSUM"))

        wt = wpool.tile([128, TOTW], BF, name="wt")
        nc.sync.dma_start(wt[:], wtp[:])
        bi = wt[:, BIAS_C:TOTW].bitcast(F32)          # [128, 19] fp32 view

        outbuf = opool.tile([128, HALF], BF, name="outbuf")

        def mm(out_ap, lhsT, rhs, start, stop, tp=None):
            for h in (0, H):
                nc.tensor.matmul(out_ap[:, h:h + H], lhsT, rhs[:, h:h + H],
                                 start=start, stop=stop, tile_position=tp,
                                 skip_group_check=True)

        state = {}

        def front(u):
            b, p = divmod(u, 2)
            hf, lb = divmod(b, 4)
            if u % 8 == 0:
                xt = inpool.tile([128, 4 * F], BF, tag="xin", name=f"xin{hf}")
                nc.sync.dma_start(xt[:], xin[:, hf * 4 * F:(hf + 1) * 4 * F])
                state["xt"] = xt
            xc = state["xt"][:, lb * F:(lb + 1) * F]
            rA, rB = 64 * p, 64 * p + 32

            def tpr(row, col):
                return (row, col) if row == 96 else None

            ps_z0 = pspool.tile([128, F], F32, tag="ps", name=f"psz0_{u}")
            mm(ps_z0[0:64, :], wt[rA:rA + 16, MM1_C:MM1_C + 64], xc[rA:rA + 16, :],
               True, True, tpr(rA, 0))
            mm(ps_z0[64:128, :], wt[rB:rB + 16, MM1_C:MM1_C + 64], xc[rB:rB + 16, :],
               True, True, tpr(rB, 64))
            ps_cc = pspool.tile([128, F], F32, tag="ps", name=f"pscc{u}")
            mm(ps_cc[0:64, :], wt[rA:rA + 16, MM1_C + 64:MM1_C + 128],
               xc[rA:rA + 16, :], True, True, tpr(rA, 0))
            mm(ps_cc[64:128, :], wt[rB:rB + 16, MM1_C + 64:MM1_C + 128],
               xc[rB:rB + 16, :], True, True, tpr(rB, 64))

            e0 = epool.tile([128, F], F32, tag="e", name=f"e0_{u}")
            nc.scalar.activation(e0[:], ps_z0[:], AF.Exp, bias=bi[:, B_Z0:B_Z0 + 1])
            zt = [zpool.tile([128, F], BF, tag=f"z{k}", name=f"zt{u}_{k}")
                  for k in range(L)]
            nc.scalar.activation(zt[0][:], e0[:], AF.Ln, bias=bi[:, B_ONE:B_ONE + 1])

            em = epool.tile([128, F], F32, tag="e", name=f"em{u}")
            nc.scalar.activation(em[:], ps_cc[:], AF.Exp, bias=bi[:, B_CC:B_CC + 1])
            em2 = mpool.tile([128, F], F32, tag="m", name=f"em2_{u}")
            nc.vector.tensor_scalar(em2[:], em[:], 1.0, None, OP.min)
            ccp = mpool.tile([128, F], BF, tag="ccp", name=f"ccp{u}")
            nc.vector.scalar_tensor_tensor(ccp[:], ps_cc[:], bi[:, B_CC1:B_CC1 + 1],
                                           em2[:], OP.add, OP.max)

            ps_repA = pspool.tile([128, F], F32, tag="ps", name=f"psra{u}")
            mm(ps_repA[:, :], wt[rA:rA + 16, REPA_C:REPA_C + 128],
               xc[rA:rA + 16, :], True, True, tpr(rA, 0))
            ps_cxA = pspool.tile([128, F], F32, tag="ps", name=f"pscxa{u}")
            mm(ps_cxA[:, :], wt[0:64, CX_C:CX_C + 128], ccp[0:64, :], True, True)
            cxa = mpool.tile([128, F], BF, tag="cxa", name=f"cxa{u}")
            nc.vector.tensor_scalar(cxa[:], ps_cxA[:], bi[:, B_CXA:B_CXA + 1],
                                    None, OP.add)
            xxa = mpool.tile([128, F], BF, tag="xxa", name=f"xxa{u}")
            nc.vector.tensor_mul(xxa[:], cxa[:], ps_repA[:])

            ps_repB = pspool.tile([128, F], F32, tag="ps", name=f"psrb{u}")
            mm(ps_repB[:, :], wt[rB:rB + 16, REPB_C:REPB_C + 128],
               xc[rB:rB + 16, :], True, True, tpr(rB, 0))
            ps_cxB = pspool.tile([128, F], F32, tag="ps", name=f"pscxb{u}")
            mm(ps_cxB[:, :], wt[64:128, CX_C:CX_C + 128], ccp[64:128, :], True, True)
            cxb = mpool.tile([128, F], BF, tag="cxb", name=f"cxb{u}")
            nc.vector.tensor_scalar(cxb[:], ps_cxB[:], bi[:, B_CXB:B_CXB + 1],
                                    None, OP.add)
            xxb = mpool.tile([128, F], BF, tag="xxb", name=f"xxb{u}")
            nc.vector.tensor_mul(xxb[:], cxb[:], ps_repB[:])

            st = dict(u=u, zt=zt, ccp=ccp, qts={})
            yield st

            czt = []
            for ci, (j, k) in enumerate(CHUNKS):
                ps_c = pspool.tile([128, F], F32, tag="ps", name=f"psc{u}_{ci}")
                co = CZ_C + 64 * ci
                mm(ps_c[0:64, :], wt[0:64, co:co + 64], ccp[0:64, :], True, True)
                mm(ps_c[64:128, :], wt[64:128, co:co + 64], ccp[64:128, :],
                   True, True)
                ec = epool.tile([128, F], F32, tag="e", name=f"ec{u}_{ci}")
                nc.scalar.activation(ec[:], ps_c[:], AF.Exp,
                                     bias=bi[:, B_CZ + ci:B_CZ + ci + 1])
                ct = czpool.tile([128, F], BF, tag=f"c{ci}", name=f"ct{u}_{ci}")
                nc.scalar.activation(ct[:], ec[:], AF.Ln,
                                     bias=bi[:, B_ONE:B_ONE + 1])
                czt.append(ct)
                if ci in (2, 5):
                    yield

            # products against z0 (ready now) for all stages
            qts = st["qts"]
            for j in range(L):
                ci = CIDX[(j, 0)]
                qt = qpool.tile([128, F], BF, tag=f"q{ci}", name=f"qt{u}_{j}_0")
                nc.vector.tensor_mul(qt[:], zt[0][:], czt[ci][:])
                qts[(j, 0)] = qt
            st["czt"] = czt
            st["xxa"] = xxa
            st["xxb"] = xxb
            yield

        def back(st):
            u, zt, czt, ccp, xxa, xxb, qts = (st["u"], st["zt"], st["czt"],
                                              st["ccp"], st["xxa"], st["xxb"],
                                              st["qts"])
            for j in range(L):
                yield
                M = 64 if j < L - 1 else 1
                ps_pre = pspool.tile([128, F], F32, tag="ps", name=f"pspre{u}_{j}")
                outA = ps_pre[0:M, :]
                outB = ps_pre[64:64 + M, :]
                ra, rb = 32 * j, 32 * SJ[j]
                mm(outA, wt[ra:ra + 8, XA_C:XA_C + M], xxa[ra:ra + 8, :],
                   True, False, (96, 0) if ra == 96 else None)
                mm(outB, wt[rb:rb + 8, XB_C:XB_C + M], xxb[rb:rb + 8, :],
                   True, False, (96, 64) if rb == 96 else None)
                for k in range(j + 1):
                    qt = qts[(j, k)]
                    off = _pre_off(j, k)
                    last = (j == L - 1) and (k == j)
                    mm(outA, wt[0:64, off:off + M], qt[0:64, :], False, last)
                    mm(outB, wt[64:128, off:off + M], qt[64:128, :], False, last)
                if j < L - 1:
                    wo = _wcc_off(j)
                    mm(outA, wt[0:64, wo:wo + 64], ccp[0:64, :], False, True)
                    mm(outB, wt[64:128, wo:wo + 64], ccp[64:128, :], False, True)
                    ej = epool.tile([128, F], F32, tag="e", name=f"ej{u}_{j}")
                    nc.scalar.activation(ej[:], ps_pre[:], AF.Exp,
                                         bias=bi[:, B_ZJ + j:B_ZJ + j + 1])
                    nc.scalar.activation(zt[j + 1][:], ej[:], AF.Ln,
                                         bias=bi[:, B_ONE:B_ONE + 1])
                    # products against the fresh z_{j+1} for later stages
                    for j2 in range(j + 1, L):
                        ci = CIDX[(j2, j + 1)]
                        qt = qpool.tile([128, F], BF, tag=f"q{ci}",
                                        name=f"qt{u}_{j2}_{j + 1}")
                        nc.vector.tensor_mul(qt[:], zt[j + 1][:], czt[ci][:])
                        qts[(j2, j + 1)] = qt
                else:
                    nc.vector.tensor_scalar(outbuf[:, u * F:(u + 1) * F], ps_pre[:],
                                            bi[:, B_FIN:B_FIN + 1], None, OP.add)

        prev = None
        for u in range(UNITS):
            fgen = front(u)
            cur = next(fgen)          # runs part 1, returns state
            bgen = back(prev) if prev is not None else iter(())
            # interleave: B-stage then F-part, so ACT fillers sit behind
            # each chain stage in the engine FIFOs
            done_b = prev is None
            while True:
                if not done_b:
                    try:
                        next(bgen)
                    except StopIteration:
                        done_b = True
                try:
                    next(fgen)
                except StopIteration:
                    if done_b:
                        break
            prev = cur
        for _ in back(prev):
            pass

        nc.sync.dma_start(out2[:], outbuf[:])
    _compile_with_pinned_act_tables(nc)
    return nc


def _compile_with_pinned_act_tables(nc):
    """bacc's insert_act_table_loads greedily picks the first table set
    containing each activation function, which alternates Exp->exp_and_others
    and Ln->natural_log, reloading tables (~1.3us) on nearly every ACTIVATE.
    Restrict Exp/Ln to the combined set (contents only -- set ids/order are
    untouched) so one load serves the whole kernel, then restore."""
    import concourse.mybir as _mb
    orig = bacc.get_activation_tables

    def patched(arch):
        tabs = orig(arch)
        out = {}
        for name, funcs in tabs.items():
            if name != "natural_log_exp_and_others":
                funcs = {f for f in funcs
                         if f not in (_mb.ActivationFunctionType.Exp,
                                      _mb.ActivationFunctionType.Ln)}
            out[name] = funcs
        return out

    bacc.get_activation_tables = patched
    try:
        nc.compile()
    finally:
        bacc.get_activation_tables = orig


_CACHED = {}


def _get_bass():
    if "nc" not in _CACHED:
        _CACHED["nc"] = build_bass()
    return _CACHED["nc"]


def run(inputs, trace=False):
    """Returns (out [N, 1] fp32, BassKernelResults)."""
    wts = fold_weights(inputs)
    xins = pack_inputs(inputs["x"], inputs["c"])
    nc = _get_bass()
    in_maps = [dict(xin=xins[core], wts=wts) for core in range(NCORES)]
    res = run_bass_kernel_spmd(nc, in_maps, list(range(NCORES)), trace=trace)
    outs = []
    for core in range(NCORES):
        o2 = res.results[core]["out2"]                     # [128, HALF] bf16
        a = np.asarray(o2[0]).astype(np.float32).reshape(UNITS, F)
        bb = np.asarray(o2[64]).astype(np.float32).reshape(UNITS, F)
        o = np.stack([a, bb], axis=1).reshape(NC)          # [units, 2, F]
        outs.append(o)
    return np.concatenate(outs).reshape(N, 1).astype(np.float32), res


def kernel(**inputs):
    out, _ = run(inputs, trace=False)
    return out


# revision 13
# speedup vs baseline: 1.1516x; 1.0007x over previous
"""DensePICNN forward pass on 8 TRN2 NeuronCores (Bass/Tile).

Strategy
--------
Pure data parallel: batch N=262144 split 8 ways (32768/core); weights
replicated. Within a core, samples run feature-major in "pair-units" of
2048: half A on SBUF partitions 0-63, half B on 64-127, so elementwise ops
use all 128 lanes and K=64 matmuls row-pack A/B into the 128-row PE array.

Host folds (fp64): actnorm scales, softplus(W) of PosLinear, gains, bias
shifts, the elu(cc)+1 reparameterization (cc' = cc+1; downstream biases get
-rowsum(W) corrections). Matmul operands bf16, PSUM fp32.

softplus(v) = Ln(Exp(v + b) + 1)   (one ACT table set)
elu(a) + 1  = max(a + 1, exp(min(a, 0)))   (exact, by convexity)

x-replication for the (x*cx) products is done on-device by scatter matmuls
(REP blocks) so the input stream stays small. Exactly 4 DMAs total (weights,
2 input halves, 1 output) keeps the kernel-tail drain within the 8-wait ISA
limit (3 engine sems + 4 DMA lanes = 7).
"""
import numpy as np
import ml_dtypes
from contextlib import ExitStack

import concourse.bass as bass
import concourse.tile as tile
from concourse import bacc
import concourse.mybir as mybir
from concourse.bass_utils import run_bass_kernel_spmd

AF = mybir.ActivationFunctionType
OP = mybir.AluOpType
BF = mybir.dt.bfloat16
F32 = mybir.dt.float32
bf16 = ml_dtypes.bfloat16

# ---- problem constants (kernel.py must be self-contained) ----
N, DIM, DIMC, DIMH, L = 262144, 8, 8, 64, 4
NCORES = 8
NC = N // NCORES            # 32768 samples per core
F = 1024                    # free-dim columns per pair-unit
H = F // 2                  # psum-bank column split
UNITS = NC // (2 * F)       # 16 pair-units per core
HALF = NC // 2              # 16384 = UNITS * F

CZ_SHIFT0 = float(np.exp(-1.0))
CZ_SHIFT_LAST = float(np.log(np.e - 1.0))

CHUNKS = [(j, k) for j in range(L) for k in range(j + 1)]      # 10 (j, k)
CIDX = {jk: i for i, jk in enumerate(CHUNKS)}
SJ = [2, 3, 0, 1]           # B-side xx/xcx row-group stagger: row 32*SJ[j]

# wts column layout (bf16)
MM1_C = 0                   # [128] z0 cols 0:64, cc cols 64:128 (4 row-bases)
CZ_C = 128                  # [640] 10 chunks of 64
CX_C = 768                  # [128] cx block, M=128 sparse
XA_C = 896                  # [64]  Wx lhsT, rows 32j+r
XB_C = 960                  # [64]  Wx lhsT, rows 32*SJ[j]+r
PRE_C = 1024                # [580] pre-chain pos/wcc blocks
REPA_C = 1604               # [128] x-scatter for A (rows {0-7, 64-71})
REPB_C = 1732               # [128] x-scatter for B (rows {32-39, 96-103})
BIAS_C = 1860               # [42]  21 fp32 bias columns, bitcast view
TOTW = BIAS_C + 42

# bias column indices (fp32 view)
B_Z0, B_CC, B_CC1 = 0, 1, 2
B_CZ = 3                    # 3..12
B_ZJ = 13                   # 13..15 (z1..z3)
B_FIN = 16
B_CXA, B_CXB = 17, 18
B_ONE, B_ZERO = 19, 20
BCOLS = 21


def _pre_off(j, k):
    base = {0: 0, 1: 128, 2: 320, 3: 576}[j]
    return PRE_C + base + (64 * k if j < 3 else k)


def _wcc_off(j):
    return PRE_C + {0: 64, 1: 256, 2: 512}[j]


def fold_weights(inp):
    """Host fp64 folding -> single packed [128, TOTW] bf16 array (biases
    stored bitwise as fp32 in the last 38 bf16 columns)."""
    g = {k: (np.asarray(v, np.float64) if not isinstance(v, list)
             else [np.asarray(a, np.float64) for a in v]) for k, v in inp.items()}
    sp = lambda v: np.logaddexp(0, v)

    wts = np.zeros((128, BIAS_C), np.float64)
    bias = np.zeros((128, BCOLS), np.float64)

    s0 = np.exp(g['an0_w'])
    W1z = g['Wz_ws'][0] * s0[:, None]
    b1z = g['Wz_bs'][0] * s0 + g['an0_b']
    sc = np.exp(g['anc_w'])
    W1c = g['Wc_w'] * sc[:, None]
    b1c = g['Wc_b'] * sc + g['anc_b']

    for base in (0, 32, 64, 96):
        wts[base + 0:base + 8, MM1_C + 0:MM1_C + 64] = W1z.T
        wts[base + 8:base + 16, MM1_C + 64:MM1_C + 128] = W1c.T
    for half in (slice(0, 64), slice(64, 128)):
        bias[half, B_Z0] = b1z
        bias[half, B_CC] = b1c
        bias[half, B_CC1] = b1c + 1.0

    # REP scatter blocks: psum row 32j+r (A) / 32*SJ[j]+r (B) <- x feat r
    for j in range(L):
        for r in range(DIM):
            for base in (0, 64):
                wts[base + r, REPA_C + 32 * j + r] = 1.0
            for base in (32, 96):
                wts[base + r, REPB_C + 32 * SJ[j] + r] = 1.0

    # cz chunks
    for (j, k) in CHUNKS:
        ci = CIDX[(j, k)]
        Wcz = g['Wcz_ws'][j]
        shift = CZ_SHIFT0 if j < L - 1 else CZ_SHIFT_LAST
        bcz = g['Wcz_bs'][j] + shift - Wcz.sum(1)
        Wch = Wcz[64 * k:64 * (k + 1), :]
        for half in (slice(0, 64), slice(64, 128)):
            wts[half, CZ_C + 64 * ci:CZ_C + 64 * (ci + 1)] = Wch.T
            bias[half, B_CZ + ci] = bcz[64 * k:64 * (k + 1)]

    # cx block + bcx bias columns
    for j in range(L):
        Wcx = g['Wcx_ws'][j]
        bcx = g['Wcx_bs'][j] + 1.0 - Wcx.sum(1)
        ra, rb = 32 * j, 32 * SJ[j]
        for r in range(DIM):
            wts[0:64, CX_C + ra + r] = Wcx[r, :]
            wts[64:128, CX_C + rb + r] = Wcx[r, :]
            bias[ra + r, B_CXA] = bcx[r]
            bias[rb + r, B_CXB] = bcx[r]

    # pre-chain weights
    for j in range(L):
        if j < L - 1:
            sjx = np.exp(g['an_ws'][j])
            gain = 1.0 / (DIMH * (j + 1))
            Wp = sp(g['Wz_ws'][j + 1]) * gain * sjx[:, None]
            Wxx = g['Wx_ws'][j] * sjx[:, None]
            Wcc = g['Wcc_ws'][j] * sjx[:, None]
            bpre = ((g['Wz_bs'][j + 1] * gain + g['Wx_bs'][j] + g['Wcc_bs'][j]
                     - g['Wcc_ws'][j].sum(1)) * sjx + g['an_bs'][j])
            for half in (slice(0, 64), slice(64, 128)):
                bias[half, B_ZJ + j] = bpre
                wts[half, _wcc_off(j):_wcc_off(j) + 64] = Wcc.T
        else:
            sL = np.exp(g['an_ws'][L - 1])
            gain = 1.0 / (DIMH * L)
            Wp = sp(g['Wz_ws'][L]) * gain * sL
            Wxx = g['Wx_ws'][L - 1] * sL
            bias[0, B_FIN] = g['an_bs'][L - 1][0]
            bias[64, B_FIN] = g['an_bs'][L - 1][0]
        M = 64 if j < L - 1 else 1
        for k in range(j + 1):
            blk = Wp[:, 64 * k:64 * (k + 1)]
            off = _pre_off(j, k)
            wts[0:64, off:off + M] = blk.T
            wts[64:128, off:off + M] = blk.T
        ra, rb = 32 * j, 32 * SJ[j]
        for r in range(DIM):
            wts[ra + r, XA_C:XA_C + M] = Wxx[:, r]
            wts[rb + r, XB_C:XB_C + M] = Wxx[:, r]

    bias[:, B_ONE] = 1.0
    packed = np.zeros((128, TOTW), bf16)
    packed[:, :BIAS_C] = wts.astype(bf16)
    packed[:, BIAS_C:] = bias.astype(np.float32).view(np.uint16).view(bf16)
    return packed


def pack_inputs(x, c):
    """Per-core xin [128, 8F] bf16: block b holds units (2b, 2b+1); unit at
    row base 64*(u%2): rows +0..7 x[A], +8..15 c[A], +32..39 x[B], +40..47 c[B].
    """
    x = np.asarray(x, np.float32).reshape(NCORES, NC, DIM)
    c = np.asarray(c, np.float32).reshape(NCORES, NC, DIMC)
    xins = []
    for core in range(NCORES):
        xin = np.zeros((128, UNITS // 2 * F), np.float32)
        for u in range(UNITS):
            b, p = divmod(u, 2)
            cols = slice(b * F, (b + 1) * F)
            s0 = u * 2 * F
            ra = 64 * p
            xin[ra + 0:ra + 8, cols] = x[core][s0:s0 + F].T
            xin[ra + 8:ra + 16, cols] = c[core][s0:s0 + F].T
            xin[ra + 32:ra + 40, cols] = x[core][s0 + F:s0 + 2 * F].T
            xin[ra + 40:ra + 48, cols] = c[core][s0 + F:s0 + 2 * F].T
        xins.append(xin.astype(bf16))
    return xins


def build_bass():
    nc = bacc.Bacc("TRN2")
    xin = nc.declare_dram_parameter("xin", [128, UNITS // 2 * F], BF, isOutput=False)
    wtp = nc.declare_dram_parameter("wts", [128, TOTW], BF, isOutput=False)
    out2 = nc.declare_dram_parameter("out2", [128, HALF], BF, isOutput=True)

    with ExitStack() as ctx:
        tc = ctx.enter_context(tile.TileContext(nc))
        wpool = ctx.enter_context(tc.tile_pool(name="w", bufs=1))
        inpool = ctx.enter_context(tc.tile_pool(name="xi", bufs=2))
        epool = ctx.enter_context(tc.tile_pool(name="e", bufs=4))
        zpool = ctx.enter_context(tc.tile_pool(name="z", bufs=2))
        czpool = ctx.enter_context(tc.tile_pool(name="cz", bufs=2))
        qpool = ctx.enter_context(tc.tile_pool(name="q", bufs=2))
        mpool = ctx.enter_context(tc.tile_pool(name="m", bufs=2))
        opool = ctx.enter_context(tc.tile_pool(name="o", bufs=1))
        pspool = ctx.enter_context(tc.tile_pool(name="ps", bufs=4, space="PSUM"))

        wt = wpool.tile([128, TOTW], BF, name="wt")
        nc.sync.dma_start(wt[:], wtp[:])
        bi = wt[:, BIAS_C:TOTW].bitcast(F32)          # [128, 19] fp32 view

        outbuf = opool.tile([128, HALF], BF, name="outbuf")

        def mm(out_ap, lhsT, rhs, start, stop, tp=None):
            for h in (0, H):
                nc.tensor.matmul(out_ap[:, h:h + H], lhsT, rhs[:, h:h + H],
                                 start=start, stop=stop, tile_position=tp,
                                 skip_group_check=True)

        state = {}

        def front(u):
            b, p = divmod(u, 2)
            hf, lb = divmod(b, 4)
            if u % 8 == 0:
                xt = inpool.tile([128, 4 * F], BF, tag="xin", name=f"xin{hf}")
                nc.sync.dma_start(xt[:], xin[:, hf * 4 * F:(hf + 1) * 4 * F])
                state["xt"] = xt
            xc = state["xt"][:, lb * F:(lb + 1) * F]
            rA, rB = 64 * p, 64 * p + 32

            def tpr(row, col):
                return (row, col) if row == 96 else None

            ps_z0 = pspool.tile([128, F], F32, tag="ps", name=f"psz0_{u}")
            mm(ps_z0[0:64, :], wt[rA:rA + 16, MM1_C:MM1_C + 64], xc[rA:rA + 16, :],
               True, True, tpr(rA, 0))
            mm(ps_z0[64:128, :], wt[rB:rB + 16, MM1_C:MM1_C + 64], xc[rB:rB + 16, :],
               True, True, tpr(rB, 64))
            ps_cc = pspool.tile([128, F], F32, tag="ps", name=f"pscc{u}")
            mm(ps_cc[0:64, :], wt[rA:rA + 16, MM1_C + 64:MM1_C + 128],
               xc[rA:rA + 16, :], True, True, tpr(rA, 0))
            mm(ps_cc[64:128, :], wt[rB:rB + 16, MM1_C + 64:MM1_C + 128],
               xc[rB:rB + 16, :], True, True, tpr(rB, 64))

            e0 = epool.tile([128, F], F32, tag="e", name=f"e0_{u}")
            nc.scalar.activation(e0[:], ps_z0[:], AF.Exp, bias=bi[:, B_Z0:B_Z0 + 1])
            zt = [zpool.tile([128, F], BF, tag=f"z{k}", name=f"zt{u}_{k}")
                  for k in range(L)]
            nc.scalar.activation(zt[0][:], e0[:], AF.Ln, bias=bi[:, B_ONE:B_ONE + 1])

            em = epool.tile([128, F], F32, tag="e", name=f"em{u}")
            nc.scalar.activation(em[:], ps_cc[:], AF.Exp, bias=bi[:, B_CC:B_CC + 1])
            em2 = mpool.tile([128, F], F32, tag="m", name=f"em2_{u}")
            nc.vector.tensor_scalar(em2[:], em[:], 1.0, None, OP.min)
            ccp = mpool.tile([128, F], BF, tag="ccp", name=f"ccp{u}")
            nc.vector.scalar_tensor_tensor(ccp[:], ps_cc[:], bi[:, B_CC1:B_CC1 + 1],
                                           em2[:], OP.add, OP.max)

            ps_repA = pspool.tile([128, F], F32, tag="ps", name=f"psra{u}")
            mm(ps_repA[:, :], wt[rA:rA + 16, REPA_C:REPA_C + 128],
               xc[rA:rA + 16, :], True, True, tpr(rA, 0))
            ps_cxA = pspool.tile([128, F], F32, tag="ps", name=f"pscxa{u}")
            mm(ps_cxA[:, :], wt[0:64, CX_C:CX_C + 128], ccp[0:64, :], True, True)
            cxa = mpool.tile([128, F], BF, tag="cxa", name=f"cxa{u}")
            nc.vector.tensor_scalar(cxa[:], ps_cxA[:], bi[:, B_CXA:B_CXA + 1],
                                    None, OP.add)
            xxa = mpool.tile([128, F], BF, tag="xxa", name=f"xxa{u}")
            nc.vector.tensor_mul(xxa[:], cxa[:], ps_repA[:])

            ps_repB = pspool.tile([128, F], F32, tag="ps", name=f"psrb{u}")
            mm(ps_repB[:, :], wt[rB:rB + 16, REPB_C:REPB_C + 128],
               xc[rB:rB + 16, :], True, True, tpr(rB, 0))
            ps_cxB = pspool.tile([128, F], F32, tag="ps", name=f"pscxb{u}")
            mm(ps_cxB[:, :], wt[64:128, CX_C:CX_C + 128], ccp[64:128, :], True, True)
            cxb = mpool.tile([128, F], BF, tag="cxb", name=f"cxb{u}")
            nc.vector.tensor_scalar(cxb[:], ps_cxB[:], bi[:, B_CXB:B_CXB + 1],
                                    None, OP.add)
            xxb = mpool.tile([128, F], BF, tag="xxb", name=f"xxb{u}")
            nc.vector.tensor_mul(xxb[:], cxb[:], ps_repB[:])

            st = dict(u=u, zt=zt, ccp=ccp, qts={})
            yield st

            czt = []
            for ci, (j, k) in enumerate(CHUNKS):
                ps_c = pspool.tile([128, F], F32, tag="ps", name=f"psc{u}_{ci}")
                co = CZ_C + 64 * ci
                mm(ps_c[0:64, :], wt[0:64, co:co + 64], ccp[0:64, :], True, True)
                mm(ps_c[64:128, :], wt[64:128, co:co + 64], ccp[64:128, :],
                   True, True)
                ec = epool.tile([128, F], F32, tag="e", name=f"ec{u}_{ci}")
                nc.scalar.activation(ec[:], ps_c[:], AF.Exp,
                                     bias=bi[:, B_CZ + ci:B_CZ + ci + 1])
                ct = czpool.tile([128, F], BF, tag=f"c{ci}", name=f"ct{u}_{ci}")
                nc.scalar.activation(ct[:], ec[:], AF.Ln,
                                     bias=bi[:, B_ONE:B_ONE + 1])
                czt.append(ct)
                if ci in (2, 5, 7):
                    yield

            # products against z0 (ready now) for all stages
            qts = st["qts"]
            for j in range(L):
                ci = CIDX[(j, 0)]
                qt = qpool.tile([128, F], BF, tag=f"q{ci}", name=f"qt{u}_{j}_0")
                nc.vector.tensor_mul(qt[:], zt[0][:], czt[ci][:])
                qts[(j, 0)] = qt
            st["czt"] = czt
            st["xxa"] = xxa
            st["xxb"] = xxb
            yield

        def back(st):
            u, zt, czt, ccp, xxa, xxb, qts = (st["u"], st["zt"], st["czt"],
                                              st["ccp"], st["xxa"], st["xxb"],
                                              st["qts"])
            for j in range(L):
                yield
                M = 64 if j < L - 1 else 1
                ps_pre = pspool.tile([128, F], F32, tag="ps", name=f"pspre{u}_{j}")
                outA = ps_pre[0:M, :]
                outB = ps_pre[64:64 + M, :]
                ra, rb = 32 * j, 32 * SJ[j]
                mm(outA, wt[ra:ra + 8, XA_C:XA_C + M], xxa[ra:ra + 8, :],
                   True, False, (96, 0) if ra == 96 else None)
                mm(outB, wt[rb:rb + 8, XB_C:XB_C + M], xxb[rb:rb + 8, :],
                   True, False, (96, 64) if rb == 96 else None)
                for k in range(j + 1):
                    qt = qts[(j, k)]
                    off = _pre_off(j, k)
                    last = (j == L - 1) and (k == j)
                    mm(outA, wt[0:64, off:off + M], qt[0:64, :], False, last)
                    mm(outB, wt[64:128, off:off + M], qt[64:128, :], False, last)
                if j < L - 1:
                    wo = _wcc_off(j)
                    mm(outA, wt[0:64, wo:wo + 64], ccp[0:64, :], False, True)
                    mm(outB, wt[64:128, wo:wo + 64], ccp[64:128, :], False, True)
                    ej = epool.tile([128, F], F32, tag="e", name=f"ej{u}_{j}")
                    nc.scalar.activation(ej[:], ps_pre[:], AF.Exp,
                                         bias=bi[:, B_ZJ + j:B_ZJ + j + 1])
                    nc.scalar.activation(zt[j + 1][:], ej[:], AF.Ln,
                                         bias=bi[:, B_ONE:B_ONE + 1])
                    # products against the fresh z_{j+1} for later stages
                    for j2 in range(j + 1, L):
                        ci = CIDX[(j2, j + 1)]
                        qt = qpool.tile([128, F], BF, tag=f"q{ci}",
                                        name=f"qt{u}_{j2}_{j + 1}")
                        nc.vector.tensor_mul(qt[:], zt[j + 1][:], czt[ci][:])
                        qts[(j2, j + 1)] = qt
                else:
                    nc.vector.tensor_scalar(outbuf[:, u * F:(u + 1) * F], ps_pre[:],
                                            bi[:, B_FIN:B_FIN + 1], None, OP.add)

        prev = None
        for u in range(UNITS):
            fgen = front(u)
            cur = next(fgen)          # runs part 1, returns state
            bgen = back(prev) if prev is not None else iter(())
            # interleave: B-stage then F-part, so ACT fillers sit behind
            # each chain stage in the engine FIFOs
            done_b = prev is None
            while True:
                if not done_b:
                    try:
                        next(bgen)
                    except StopIteration:
                        done_b = True
                try:
                    next(fgen)
                except StopIteration:
                    if done_b:
                        break
            prev = cur
        for _ in back(prev):
            pass

        nc.sync.dma_start(out2[:], outbuf[:])
    _compile_with_pinned_act_tables(nc)
    return nc


def _compile_with_pinned_act_tables(nc):
    """bacc's insert_act_table_loads greedily picks the first table set
    containing each activation function, which alternates Exp->exp_and_others
    and Ln->natural_log, reloading tables (~1.3us) on nearly every ACTIVATE.
    Restrict Exp/Ln to the combined set (contents only -- set ids/order are
    untouched) so one load serves the whole kernel, then restore."""
    import concourse.mybir as _mb
    orig = bacc.get_activation_tables

    def patched(arch):
        tabs = orig(arch)
        out = {}
        for name, funcs in tabs.items():
            if name != "natural_log_exp_and_others":
                funcs = {f for f in funcs
                         if f not in (_mb.ActivationFunctionType.Exp,
                                      _mb.ActivationFunctionType.Ln)}
            out[name] = funcs
        return out

    bacc.get_activation_tables = patched
    try:
        nc.compile()
    finally:
        bacc.get_activation_tables = orig


_CACHED = {}


def _get_bass():
    if "nc" not in _CACHED:
        _CACHED["nc"] = build_bass()
    return _CACHED["nc"]


def run(inputs, trace=False):
    """Returns (out [N, 1] fp32, BassKernelResults)."""
    wts = fold_weights(inputs)
    xins = pack_inputs(inputs["x"], inputs["c"])
    nc = _get_bass()
    in_maps = [dict(xin=xins[core], wts=wts) for core in range(NCORES)]
    res = run_bass_kernel_spmd(nc, in_maps, list(range(NCORES)), trace=trace)
    outs = []
    for core in range(NCORES):
        o2 = res.results[core]["out2"]                     # [128, HALF] bf16
        a = np.asarray(o2[0]).astype(np.float32).reshape(UNITS, F)
        bb = np.asarray(o2[64]).astype(np.float32).reshape(UNITS, F)
        o = np.stack([a, bb], axis=1).reshape(NC)          # [units, 2, F]
        outs.append(o)
    return np.concatenate(outs).reshape(N, 1).astype(np.float32), res


def kernel(**inputs):
    out, _ = run(inputs, trace=False)
    return out
